# revision 25
# baseline (speedup 1.0000x reference)
"""Trainium2 Bass kernel for nn_Attention_28948079575569 (softmax pooling).

Computation (reference):
    u      = tanh(h @ W1^T + b1)                 [B, T, D]
    alphas = softmax_t(u @ W2^T)                 [B, T, D]
    out    = sum_{b,t} h * alphas                [D]

Distribution: data-parallel over batch across 8 NeuronCores (4 batches per
core); small weights replicated; each core emits a partial [D] sum which the
host adds (the cross-core reduction is 2KB — not worth a collective).

Per-core dataflow (transposed space, features on partitions). h is host-cast
to bf16 (numerically identical to the previous on-chip DMA cast) so the
DMA xbar can transpose it HBM->SBUF directly:
    h^T   via dma_start_transpose from HBM       [128, NT, TCH] bf16
    u^T   = tanh(W1 @ h^T + b1)      TensorE (lhsT = W1^T blocks) + ScalarE
    s^T   = W2 @ u^T                 TensorE (lhsT = W2^T blocks)
    P     = exp(s^T)                 ScalarE, fused accum_out -> Z (denominator)
    N     = sum_t h^T * P            VectorE fused tensor_tensor_reduce
    out   = sum_b N_b / Z_b          small VectorE epilogue per batch

exp needs no max-subtraction: |s| <= ||u||*||W2_row|| is bounded (~26 worst
case since |u|<1 via tanh), far below f32 overflow.
"""
import numpy as np
import ml_dtypes

import concourse.bacc as bacc
import concourse.bass as bass
import concourse.tile as tile
from concourse import bass_utils, mybir

F32 = mybir.dt.float32
BF16 = mybir.dt.bfloat16
Act = mybir.ActivationFunctionType
Alu = mybir.AluOpType

B, T, D = 32, 4096, 512
N_CORES = 8
B_LOC = B // N_CORES      # batches per core
TL = B_LOC * T            # rows per core
TCH = 512                 # t-chunk size
NCH = T // TCH            # chunks per batch
NT = D // 128             # 128-partition tiles per feature dim


def build(repeat=1, loop=False, ab="", bufs=4, ct=2, psum_bufs=8,
          xb_eng="alt", fuse_num=False, batched_epi=True, pair_exp=False,
          ilv=True):
    # pair_exp=True (2-bank PSUM pairs + batched exp) measured WORSE
    # (+27us): splitting the shared 8-slot PSUM pool into 4+2x2 loses more
    # rotation slack than the two saved ACT instruction overheads gain.
    # fuse_num=True (vector.tensor_tensor_reduce) hangs the device on this
    # runtime — keep the separate mul+reduce; the numerator is fully hidden
    # under TensorE anyway (measured 0 marginal cost).
    # ct: col-tiling factor via tile_position (2 = two concurrent M=64
    #     matmuls so LDWEIGHTS of one col-group overlaps the other; measured
    #     best in the earlier PE-transpose design).
    # ab: timing-only ablations ("nonum" drop numerator, "nomm2" shrink mm2,
    #     "notr" replace the transposed load with a plain load of the same
    #     bytes). Results become WRONG; only for time attribution.
    nc = bacc.Bacc("TRN2", target_bir_lowering=False, debug=False)
    h = nc.dram_tensor("h", [TL, D], BF16, kind="ExternalInput")
    w1t = nc.dram_tensor("w1t", [D, D], BF16, kind="ExternalInput")  # W1.T
    w2t = nc.dram_tensor("w2t", [D, D], BF16, kind="ExternalInput")  # W2.T
    b1 = nc.dram_tensor("b1", [D, 1], F32, kind="ExternalInput")
    out = nc.dram_tensor("out", [NT, 128], F32, kind="ExternalOutput")

    # h arrives HOST-PRE-TRANSPOSED: row (ch, kk, p) holds the TCH t-values
    # of feature d = kk*128 + p in chunk ch, so a plain DMA yields hT
    # directly ([128, NT, TCH], d on partitions) with 1KB-contiguous reads.
    hvt = h.ap().rearrange("(ch kk p) t -> ch p kk t", kk=NT, p=128)

    with tile.TileContext(nc) as tc:
        import contextlib
        stk = contextlib.ExitStack()
        wp = stk.enter_context(tc.tile_pool(name="wts", bufs=1))
        htp = stk.enter_context(tc.tile_pool(name="hT", bufs=bufs))
        up_ = stk.enter_context(tc.tile_pool(name="u", bufs=bufs))
        pp_ = stk.enter_context(tc.tile_pool(name="P", bufs=bufs))
        scp = stk.enter_context(tc.tile_pool(name="scr", bufs=2))
        znp = stk.enter_context(tc.tile_pool(name="zn", bufs=2))
        smp = stk.enter_context(tc.tile_pool(name="small", bufs=4))
        resp = stk.enter_context(tc.tile_pool(name="res", bufs=1))
        if pair_exp:
            psp = stk.enter_context(
                tc.tile_pool(name="ps", bufs=4, space="PSUM"))
            ps2p = stk.enter_context(
                tc.tile_pool(name="ps2", bufs=2, space="PSUM"))
        else:
            psp = stk.enter_context(
                tc.tile_pool(name="ps", bufs=psum_bufs, space="PSUM"))
        if True:
            w1_sb = wp.tile([128, NT, D], BF16)
            nc.sync.dma_start(
                w1_sb[:], w1t.ap().rearrange("(kk p) e -> p kk e", p=128))
            w2_sb = wp.tile([128, NT, D], BF16)
            nc.sync.dma_start(
                w2_sb[:], w2t.ap().rearrange("(kk p) e -> p kk e", p=128))
            b1_sb = wp.tile([128, NT, 1], F32)
            nc.sync.dma_start(
                b1_sb[:], b1.ap().rearrange("(i p) o -> p i o", p=128))

            acc = resp.tile([128, NT], F32)
            nc.vector.memset(acc[:], 0.0)

            def mm(ps, w_sb, rhs, me, kk):
                if ct == 2:
                    for j in range(2):
                        nc.tensor.matmul(
                            ps[64 * j:64 * j + 64, :],
                            w_sb[:, kk, bass.ds(me * 128 + 64 * j, 64)],
                            rhs,
                            start=(kk == 0), stop=(kk == NT - 1),
                            tile_position=(0, 64 * j),
                            skip_group_check=True)
                else:
                    nc.tensor.matmul(
                        ps[:], w_sb[:, kk, bass.ds(me * 128, 128)], rhs,
                        start=(kk == 0), stop=(kk == NT - 1))

            def stage1(ci):
                c = ci % NCH
                hT = htp.tile([128, NT, TCH], BF16, tag="hT")
                eng = (nc.sync if xb_eng == "sync"
                       else [nc.sync, nc.scalar][c % 2])
                eng.dma_start(hT[:], hvt[ci])
                # ---- mm1 + tanh ----
                u_sb = up_.tile([128, NT, TCH], BF16, tag="u")
                for me in range(NT):
                    ps = psp.tile([128, TCH], F32, tag="ps")
                    for kk in range(NT):
                        mm(ps, w1_sb, hT[:, kk, :], me, kk)
                    nc.scalar.activation(
                        u_sb[:, me, :], ps[:], Act.Tanh,
                        bias=(0.0 if ab == "nobias" else b1_sb[:, me, :]))
                return hT, u_sb

            def stage2(ci, hT, u_sb, Zc, Nc):
                c = ci % NCH
                # ---- mm2 + exp; Z via one DVE reduce over P (the ACT
                # accum_out rider measured +10.7us/iter) ----
                P_sb = pp_.tile([128, NT, TCH], BF16, tag="P")
                if pair_exp:
                    for j in range(2):
                        pr2 = ps2p.tile([128, 2, TCH], F32, tag="pp")
                        for ml in range(2):
                            me = 2 * j + ml
                            if ab == "nomm2":
                                nc.tensor.matmul(
                                    pr2[:, ml, :],
                                    w2_sb[:, 0, bass.ds(me * 128, 128)],
                                    u_sb[:, 0, :], start=True, stop=True)
                            else:
                                for kk in range(NT):
                                    mm(pr2[:, ml, :], w2_sb,
                                       u_sb[:, kk, :], me, kk)
                        nc.scalar.activation(
                            P_sb[:, 2 * j:2 * j + 2, :], pr2[:], Act.Exp)
                else:
                    for me in range(NT):
                        ps = psp.tile([128, TCH], F32, tag="ps")
                        if ab == "nomm2":
                            nc.tensor.matmul(
                                ps[:], w2_sb[:, 0, bass.ds(me * 128, 128)],
                                u_sb[:, 0, :], start=True, stop=True)
                        else:
                            for kk in range(NT):
                                mm(ps, w2_sb, u_sb[:, kk, :], me, kk)
                        nc.scalar.activation(P_sb[:, me, :], ps[:], Act.Exp)
                nc.vector.tensor_reduce(
                    Zc[:, :, c:c + 1], P_sb[:],
                    axis=mybir.AxisListType.X, op=Alu.add)
                # ---- numerator: Nc[:,me,c] = sum_t h^T * P ----
                if ab != "nonum":
                    if fuse_num:
                        for me in range(NT):
                            q = scp.tile([128, TCH], BF16, tag="sc")
                            nc.vector.tensor_tensor_reduce(
                                q[:], hT[:, me, :], P_sb[:, me, :],
                                scale=1.0, scalar=0.0,
                                op0=Alu.mult, op1=Alu.add,
                                accum_out=Nc[:, me, c:c + 1])
                    else:
                        q = scp.tile([128, NT, TCH], BF16, tag="sc")
                        nc.vector.tensor_mul(q[:], hT[:], P_sb[:])
                        nc.vector.tensor_reduce(
                            Nc[:, :, c:c + 1], q[:],
                            axis=mybir.AxisListType.X, op=Alu.add)
                elif c == 0:
                    nc.vector.memset(Nc[:], 1.0)

            def epilogue(Zc, Nc):
                # ---- batch epilogue: acc += N/Z (batched per-me ops) ----
                if batched_epi:
                    zb = smp.tile([128, NT], F32, tag="zb")
                    nc.vector.tensor_reduce(
                        zb[:], Zc[:], axis=mybir.AxisListType.X, op=Alu.add)
                    rz = smp.tile([128, NT], F32, tag="rz")
                    nc.vector.reciprocal(rz[:], zb[:])
                    nb = smp.tile([128, NT], F32, tag="nb")
                    nc.vector.tensor_reduce(
                        nb[:], Nc[:], axis=mybir.AxisListType.X, op=Alu.add)
                    pr = smp.tile([128, NT], F32, tag="pr")
                    nc.vector.tensor_mul(pr[:], nb[:], rz[:])
                    nc.vector.tensor_add(acc[:], acc[:], pr[:])
                else:
                    for me in range(NT):
                        zb = smp.tile([128, 1], F32, tag="zb")
                        nc.vector.tensor_reduce(
                            zb[:], Zc[:, me, :], axis=mybir.AxisListType.X,
                            op=Alu.add)
                        rz = smp.tile([128, 1], F32, tag="rz")
                        nc.vector.reciprocal(rz[:], zb[:])
                        nb = smp.tile([128, 1], F32, tag="nb")
                        nc.vector.tensor_reduce(
                            nb[:], Nc[:, me, :], axis=mybir.AxisListType.X,
                            op=Alu.add)
                        pr = smp.tile([128, 1], F32, tag="pr")
                        nc.vector.tensor_mul(pr[:], nb[:], rz[:])
                        nc.vector.tensor_add(
                            acc[:, me:me + 1], acc[:, me:me + 1], pr[:])

            def stage12_ilv(ci, pend):
                # me-interleaved emission: mm1(ci,me)+tanh then
                # mm2(ci-1,me)+exp, alternating — spreads ACT drains and
                # PSUM recycling evenly instead of 4+4 bursts.
                c = ci % NCH
                hT = htp.tile([128, NT, TCH], BF16, tag="hT")
                eng = (nc.sync if xb_eng == "sync"
                       else [nc.sync, nc.scalar][c % 2])
                eng.dma_start(hT[:], hvt[ci])
                u_sb = up_.tile([128, NT, TCH], BF16, tag="u")
                if pend is not None:
                    pci, phT, pu, pZc, pNc = pend
                    pc = pci % NCH
                    P_sb = pp_.tile([128, NT, TCH], BF16, tag="P")
                for me in range(NT):
                    ps = psp.tile([128, TCH], F32, tag="ps")
                    for kk in range(NT):
                        mm(ps, w1_sb, hT[:, kk, :], me, kk)
                    nc.scalar.activation(
                        u_sb[:, me, :], ps[:], Act.Tanh,
                        bias=b1_sb[:, me, :])
                    if pend is not None:
                        ps2 = psp.tile([128, TCH], F32, tag="ps")
                        for kk in range(NT):
                            mm(ps2, w2_sb, pu[:, kk, :], me, kk)
                        nc.scalar.activation(
                            P_sb[:, me, :], ps2[:], Act.Exp)
                if pend is not None:
                    nc.vector.tensor_reduce(
                        pZc[:, :, pc:pc + 1], P_sb[:],
                        axis=mybir.AxisListType.X, op=Alu.add)
                    q = scp.tile([128, NT, TCH], BF16, tag="sc")
                    nc.vector.tensor_mul(q[:], phT[:], P_sb[:])
                    nc.vector.tensor_reduce(
                        pNc[:, :, pc:pc + 1], q[:],
                        axis=mybir.AxisListType.X, op=Alu.add)
                return hT, u_sb

            def repeat_body():
                # software pipeline: stage2(ci-1) is emitted after
                # stage1(ci), so mm2 never waits on the tanh of its own
                # chunk; drains at the end of each repeat iteration.
                zn = {}
                pending = None
                for ci in range(B_LOC * NCH):
                    b, c = divmod(ci, NCH)
                    if c == 0:
                        Zt = znp.tile([128, NT, NCH], F32, tag="Zc")
                        Nt = znp.tile([128, NT, NCH], F32, tag="Nc")
                        zn[b] = (Zt, Nt)
                    if ilv:
                        pend = None
                        if pending is not None:
                            pci, phT, pu = pending
                            pend = (pci, phT, pu) + zn[pci // NCH]
                        hT, u_sb = stage12_ilv(ci, pend)
                        if pending is not None:
                            pci = pending[0]
                            if pci % NCH == NCH - 1:
                                epilogue(*zn.pop(pci // NCH))
                    else:
                        hT, u_sb = stage1(ci)
                        if pending is not None:
                            pci, phT, pu = pending
                            pb = pci // NCH
                            stage2(pci, phT, pu, *zn[pb])
                            if pci % NCH == NCH - 1:
                                epilogue(*zn.pop(pb))
                    pending = (ci, hT, u_sb)
                pci, phT, pu = pending
                pb = pci // NCH
                stage2(pci, phT, pu, *zn[pb])
                epilogue(*zn.pop(pb))

            if loop and repeat > 1:
                with tc.For_i(0, repeat, 1):
                    repeat_body()
            else:
                for _rep in range(repeat):
                    repeat_body()

            nc.sync.dma_start(out.ap().rearrange("i p -> p i"), acc[:])
            stk.close()

    nc.compile()
    return nc


def make_in_maps(hidden_states, W1, b1v, W2):
    h = np.asarray(hidden_states, dtype=np.float32).astype(ml_dtypes.bfloat16)
    W1T = np.ascontiguousarray(np.asarray(W1, np.float32).T).astype(
        ml_dtypes.bfloat16)
    W2T = np.ascontiguousarray(np.asarray(W2, np.float32).T).astype(
        ml_dtypes.bfloat16)
    b1c = np.asarray(b1v, np.float32).reshape(D, 1).copy()
    # pre-transpose per chunk: [core, b, ch, t, kk, p] -> [core, b, ch, kk, p, t]
    ht = h.reshape(N_CORES, B_LOC, NCH, TCH, NT, 128)
    ht = np.ascontiguousarray(ht.transpose(0, 1, 2, 4, 5, 3))
    hs = ht.reshape(N_CORES, TL, TCH)
    return [{"h": hs[i], "w1t": W1T, "w2t": W2T, "b1": b1c}
            for i in range(N_CORES)]


_NC_CACHE = {}


def _get_nc():
    if "nc" not in _NC_CACHE:
        _NC_CACHE["nc"] = build(repeat=1)
    return _NC_CACHE["nc"]


def kernel(hidden_states, W1, b1, W2):
    assert np.asarray(hidden_states).shape == (B, T, D)
    in_maps = make_in_maps(hidden_states, W1, b1, W2)
    nc = _get_nc()
    res = bass_utils.run_bass_kernel_spmd(
        nc, in_maps, core_ids=list(range(N_CORES)), trace=False)
    total = np.zeros(D, np.float64)
    for r in res.results:
        total += r["out"].reshape(D).astype(np.float64)
    return total.astype(np.float32)
